# revision 1
# baseline (speedup 1.0000x reference)
"""Causal self-attention (B=4, T=2048, C=1024, H=16) on 8 trn2 NeuronCores.

Sharding: core c = (batch b = c // 2, head-group hg = c % 2). Each core runs
one batch with 8 of the 16 heads: column-parallel c_attn, full causal
attention for its heads, row-parallel c_proj producing a partial [T, C]
output. The host sums the two head-group partials per batch (the row-parallel
all-reduce is folded into the unshard step).

Per-core kernel (bass/Tile):
  - x arrives pre-transposed and pre-cast on the host: xt [C, T] bf16 is
    DMA'd straight into SBUF in [c, t] layout (no on-device transposes).
  - QKV^T projection with lhsT=W (column-sliced), rhs=xT: Q^T/K^T in [d, t]
    layout. V is projected in [t, j] layout (lhsT=xT blocks).
  - Attention per head-PAIR (even head in partitions 0:64, odd in 64:128):
    the two S^T[k, q] matmuls (K=64 contraction) land on PE row-tiles T0/T8
    and run concurrently. One exp ACTIVATE covers both heads' scores
    (PSUM pair-bank tile), with the 1/sqrt(d) folded into the activation
    scale. Diagonal blocks only compute/exp the unmasked column suffix, and
    are masked in-place by a gpsimd affine_select (no mask tile, no DVE).
    att@V with lhsT=[ones | V_h]: PSUM rows 0:64 accumulate the softmax
    denominator (pre-broadcast), rows 64:128 the unnormalized output^T.
  - Emission is software-pipelined: S^T(kb) is issued before AV(kb-1), and
    projection/c_proj work units are injected into the attention stream's
    PE gaps so the tensor engine stays busy while the scalar engine
    (the exp throughput bound) paces the attention phase.
  - Normalize with reciprocal_approx_fast + multiply, c_proj from attT
    (lhsT) with row-sliced W_proj, bias adds fused into PSUM->SBUF copies.

All matmuls are bf16 with fp32 PSUM accumulation (scale-relative absmax vs
the fp32 reference ~4e-3).
"""

from collections import deque
from contextlib import ExitStack

import ml_dtypes
import numpy as np

import concourse.bass as bass
import concourse.mybir as mybir
from concourse import bacc
from concourse.bass_utils import run_bass_kernel_spmd
from concourse.tile import TileContext

F32 = mybir.dt.float32
BF16 = mybir.dt.bfloat16

P = 128
D = 64          # head dim
HG = 8          # heads per core
JQ = HG * D     # 512 j-channels per q/k/v section per core
C = 1024        # model dim
B = 4
T = 2048
KSUB = C // P   # 8
JT_Q = JQ // P  # 4
QTILE = 512
NPAIR = HG // 2
N_CORES = 8


def _build_nc(niter_loop=False):
    nc = bacc.Bacc("TRN2", target_bir_lowering=False, debug=False)

    TT = T // P
    NQT = T // QTILE

    xt = nc.dram_tensor("xt", [C, T], BF16, kind="ExternalInput")
    wqk = nc.dram_tensor("wqk", [C, 2 * JQ], BF16, kind="ExternalInput")
    wv = nc.dram_tensor("wv", [C, JQ], BF16, kind="ExternalInput")
    wp = nc.dram_tensor("wp", [JQ, C], BF16, kind="ExternalInput")
    battn = nc.dram_tensor("battn", [3 * JQ], F32, kind="ExternalInput")
    bproj = nc.dram_tensor("bproj", [C], F32, kind="ExternalInput")
    out = nc.dram_tensor("out", [T, C], F32, kind="ExternalOutput")
    if niter_loop:
        niter = nc.dram_tensor("niter", [1, 1], mybir.dt.uint32, kind="ExternalInput")

    with TileContext(nc) as tc, ExitStack() as ctx:
        if niter_loop:
            niter_pool = ctx.enter_context(tc.tile_pool(name="niter", bufs=1))
            niter_sb = niter_pool.tile([1, 1], mybir.dt.uint32)
            nc.sync.dma_start(niter_sb[:], niter[:, :])
            niter_val = nc.values_load(
                niter_sb[0:1, 0:1],
                min_val=0,
                max_val=1 << 20,
                skip_runtime_bounds_check=True,
            )
            loop_cm = tc.For_i(0, niter_val)
            ctx.enter_context(loop_cm)

        consts = ctx.enter_context(tc.tile_pool(name="consts", bufs=1))
        wpool = ctx.enter_context(tc.tile_pool(name="wpool", bufs=1))
        big = ctx.enter_context(tc.tile_pool(name="big", bufs=1))

        battn_sb = consts.tile([P, 3 * JQ // P], F32)
        nc.sync.dma_start(battn_sb[:], battn.rearrange("(a p) -> p a", p=P))
        bv_row = consts.tile([1, JQ], F32)
        nc.sync.dma_start(bv_row[:], battn[None, 2 * JQ:])
        bv_bc = consts.tile([P, JQ], F32)
        nc.gpsimd.partition_broadcast(bv_bc[:], bv_row[:])
        bp_row = consts.tile([1, C], F32)
        nc.sync.dma_start(bp_row[:], bproj[None, :])
        bp_bc = consts.tile([P, C], F32)
        nc.gpsimd.partition_broadcast(bp_bc[:], bp_row[:])

        # weights: split wqk DMA into q/k halves so QK units start early
        wqk_sb = wpool.tile([P, KSUB, 2 * JQ], BF16)
        wqk_r = wqk.rearrange("(ko ki) j -> ki ko j", ki=P)
        nc.sync.dma_start(wqk_sb[:, :, 0:JQ], wqk_r[:, :, 0:JQ])
        nc.sync.dma_start(wqk_sb[:, :, JQ:], wqk_r[:, :, JQ:])
        wv_sb = wpool.tile([P, KSUB, JQ], BF16)
        nc.sync.dma_start(wv_sb[:], wv.rearrange("(ko ki) j -> ki ko j", ki=P))
        wp_sb = wpool.tile([P, JQ // P, C], BF16)
        nc.sync.dma_start(wp_sb[:], wp.rearrange("(ko ki) j -> ki ko j", ki=P))

        xT = big.tile([P, KSUB, T], BF16)
        for ks in range(KSUB):
            nc.sync.dma_start(xT[:, ks, :], xt[ks * P : (ks + 1) * P, :])

        QT = big.tile([P, JT_Q, T], BF16)
        KT = big.tile([P, JT_Q, T], BF16)
        V = big.tile([P, TT, HG, 2, D], BF16)
        nc.vector.memset(V[:, :, :, 0, :], 1.0)

        with tc.tile_pool(name="bpp", bufs=2, space="PSUM") as bpp, \
             tc.tile_pool(name="stp", bufs=2, space="PSUM") as stp, \
             tc.tile_pool(name="avp", bufs=1, space="PSUM") as avp, \
             tc.tile_pool(name="etp", bufs=3) as etp, \
             tc.tile_pool(name="rcp", bufs=2) as rcp, \
             tc.tile_pool(name="attw", bufs=2) as attw, \
             tc.tile_pool(name="ostage", bufs=3) as ostage:

            # ---- deferred work units (fill PE gaps in the attention phase)
            def emit_v_unit(tt):
                def go():
                    ps = bpp.tile([P, JQ], F32, tag="bp", name=f"vps{tt}")
                    for ks in range(KSUB):
                        nc.tensor.matmul(
                            ps[:],
                            lhsT=xT[:, ks, tt * P : (tt + 1) * P],
                            rhs=wv_sb[:, ks, :],
                            start=(ks == 0),
                            stop=(ks == KSUB - 1),
                        )
                    nc.vector.tensor_add(V[:, tt, :, 1, :], ps[:], bv_bc[:])
                return go

            def emit_qk_unit(jt, tq):
                def go():
                    dst = QT if jt < JT_Q else KT
                    js = jt % JT_Q
                    ps = bpp.tile([P, QTILE], F32, tag="bp", name=f"qkps{jt}_{tq}")
                    for ks in range(KSUB):
                        nc.tensor.matmul(
                            ps[:],
                            lhsT=wqk_sb[:, ks, jt * P : (jt + 1) * P],
                            rhs=xT[:, ks, tq * QTILE : (tq + 1) * QTILE],
                            start=(ks == 0),
                            stop=(ks == KSUB - 1),
                        )
                    nc.vector.tensor_scalar_add(
                        dst[:, js, tq * QTILE : (tq + 1) * QTILE],
                        ps[:],
                        battn_sb[:, jt : jt + 1],
                    )
                return go

            def emit_cproj_unit(attT_t, tl, tt, ot):
                def go():
                    for nt in range(C // QTILE):
                        ps = bpp.tile([P, QTILE], F32, tag="bp", name=f"cpps{tt}_{nt}")
                        for js2 in range(JT_Q):
                            nc.tensor.matmul(
                                ps[:],
                                lhsT=attT_t[:, js2, tl * P : (tl + 1) * P],
                                rhs=wp_sb[:, js2, nt * QTILE : (nt + 1) * QTILE],
                                start=(js2 == 0),
                                stop=(js2 == JT_Q - 1),
                            )
                        nc.vector.tensor_add(
                            ot[:, nt * QTILE : (nt + 1) * QTILE],
                            ps[:],
                            bp_bc[:, nt * QTILE : (nt + 1) * QTILE],
                        )
                    nc.sync.dma_start(out[tt * P : (tt + 1) * P, :], ot[:])
                return go

            # pending B(tq) units, keyed for just-in-time draining
            pending = [dict() for _ in range(NQT)]
            for tq in range(NQT):
                order = []
                for pair in range(NPAIR):
                    order.append(("qk", pair, tq))
                    order.append(("qk", JT_Q + pair, tq))
                    if pair == 0:
                        for tt in range(4 * tq, 4 * tq + 4):
                            order.append(("v", tt))
                for key in order:
                    if key[0] == "qk":
                        pending[tq][key] = emit_qk_unit(key[1], key[2])
                    else:
                        pending[tq][key] = emit_v_unit(key[1])

            cproj_q = deque()

            def need(tq, key):
                fn = pending[tq].pop(key, None)
                if fn is not None:
                    fn()

            def pop_fill(qt):
                # overdue first, then next tq's units, then deferred c_proj
                for tq in (qt, qt + 1):
                    if tq < NQT and pending[tq]:
                        k = next(iter(pending[tq]))
                        pending[tq].pop(k)()
                        return True
                if cproj_q:
                    cproj_q.popleft()()
                    return True
                return False

            # prime: minimal B(0) work for (qt=0, pair=0)
            need(0, ("qk", 0, 0))
            need(0, ("qk", JT_Q, 0))
            for tt in range(4):
                need(0, ("v", tt))

            # ---- attention phase, paced by the scalar-engine exp stream
            for qt in range(NQT):
                nkb = 4 * (qt + 1)
                attT = attw.tile([P, JT_Q, QTILE], BF16, tag="attT", name=f"attT{qt}")
                for pair in range(NPAIR):
                    need(qt, ("qk", pair, qt))
                    need(qt, ("qk", JT_Q + pair, qt))
                    for tt in range(nkb):
                        need(tq=tt // 4, key=("v", tt))

                    av = avp.tile([P, 2, QTILE], F32, tag="av", name=f"av{qt}_{pair}")
                    prev = None
                    for kb in range(nkb):
                        dj = kb - 4 * qt
                        q0 = P * dj if dj > 0 else 0
                        qs = slice(qt * QTILE + q0, (qt + 1) * QTILE)
                        st = stp.tile([P, 2, QTILE], F32, tag="st", name=f"st{kb}")
                        for hp in range(2):
                            r0 = hp * D
                            nc.tensor.matmul(
                                st[:, hp, q0:],
                                lhsT=KT[r0 : r0 + D, pair, kb * P : (kb + 1) * P],
                                rhs=QT[r0 : r0 + D, pair, qs],
                                start=True,
                                stop=True,
                            )
                        et = etp.tile([P, 2, QTILE], BF16, tag="et", name=f"et{kb}")
                        nc.scalar.activation(
                            et[:, :, q0:],
                            st[:, :, q0:],
                            mybir.ActivationFunctionType.Exp,
                            scale=float(1.0 / np.sqrt(D)),
                        )
                        if dj >= 0:
                            nc.gpsimd.affine_select(
                                out=et[:, :, q0 : q0 + P],
                                in_=et[:, :, q0 : q0 + P],
                                compare_op=mybir.AluOpType.is_ge,
                                fill=0.0,
                                base=0,
                                pattern=[[0, 2], [1, P]],
                                channel_multiplier=-1,
                            )
                        if kb % 2 == 1:
                            pop_fill(qt)
                        if prev is not None:
                            pkb, pet, pq0 = prev
                            for hp in range(2):
                                nc.tensor.matmul(
                                    av[:, hp, pq0:],
                                    lhsT=V[:, pkb, 2 * pair + hp],
                                    rhs=pet[:, hp, pq0:],
                                    start=(pkb == 0),
                                    stop=(pkb == nkb - 1),
                                )
                        prev = (kb, et, q0)
                    pkb, pet, pq0 = prev
                    for hp in range(2):
                        nc.tensor.matmul(
                            av[:, hp, pq0:],
                            lhsT=V[:, pkb, 2 * pair + hp],
                            rhs=pet[:, hp, pq0:],
                            start=(pkb == 0),
                            stop=(pkb == nkb - 1),
                        )
                    for hp in range(2):
                        rc = rcp.tile([D, QTILE], F32, tag="rc", name=f"rc{pair}")
                        nc.vector.reciprocal_approx_fast(rc[:], av[:D, hp, :])
                        nc.vector.tensor_mul(
                            attT[hp * D : (hp + 1) * D, pair, :], av[D:, hp, :], rc[:]
                        )

                # defer this q-tile's c_proj into the next q-tile's PE gaps
                for tl in range(QTILE // P):
                    tt = qt * (QTILE // P) + tl
                    ot = ostage.tile([P, C], F32, tag="ot", name=f"ot{tt}")
                    cproj_q.append(emit_cproj_unit(attT, tl, tt, ot))

            while pop_fill(NQT - 1):
                pass

    nc.compile()
    return nc


_NC_CACHE = {}


def _get_nc():
    if "nc" not in _NC_CACHE:
        _NC_CACHE["nc"] = _build_nc()
    return _NC_CACHE["nc"]


def _core_inputs(x, W_attn, b_attn, W_proj, b_proj, b, hg):
    bf = ml_dtypes.bfloat16
    qs = slice(hg * JQ, (hg + 1) * JQ)
    ks = slice(C + hg * JQ, C + (hg + 1) * JQ)
    vs = slice(2 * C + hg * JQ, 2 * C + (hg + 1) * JQ)
    return {
        "xt": np.ascontiguousarray(x[b].T).astype(bf),
        "wqk": np.ascontiguousarray(
            np.concatenate([W_attn[:, qs], W_attn[:, ks]], axis=1)
        ).astype(bf),
        "wv": np.ascontiguousarray(W_attn[:, vs]).astype(bf),
        "wp": np.ascontiguousarray(W_proj[hg * JQ : (hg + 1) * JQ, :]).astype(bf),
        "battn": np.ascontiguousarray(
            np.concatenate([b_attn[qs], b_attn[ks], b_attn[vs]])
        ).astype(np.float32),
        "bproj": np.asarray(b_proj, dtype=np.float32),
    }


def kernel(x, W_attn, b_attn, W_proj, b_proj):
    x = np.asarray(x, dtype=np.float32)
    W_attn = np.asarray(W_attn, dtype=np.float32)
    b_attn = np.asarray(b_attn, dtype=np.float32)
    W_proj = np.asarray(W_proj, dtype=np.float32)
    b_proj = np.asarray(b_proj, dtype=np.float32)

    nc = _get_nc()
    in_maps = [
        _core_inputs(x, W_attn, b_attn, W_proj, b_proj, b=c // 2, hg=c % 2)
        for c in range(N_CORES)
    ]
    res = run_bass_kernel_spmd(nc, in_maps, core_ids=list(range(N_CORES)))
    out = np.empty((B, T, C), dtype=np.float32)
    for b in range(B):
        out[b] = res.results[2 * b]["out"] + res.results[2 * b + 1]["out"]
    return out

